# revision 1
# baseline (speedup 1.0000x reference)
"""Trainium2 Bass kernel for the DeepSeek-V4 indexer compressor (prefill).

Contract: kernel(**inputs) takes the FULL unsharded inputs (numpy) and
returns the FULL [1, 2048, 128] float32 output.

Strategy (8 NeuronCores, sequence-parallel):
  - Each core handles 1024 tokens = 256 compress blocks, plus a 4-token
    halo from the previous core (redundantly computed via tiny N=4
    matmuls interleaved into the first chunk's stream) for the overlap
    transform.
  - On-device layout is channel-major: the contraction dim (7168) lives
    on SBUF partitions, tokens on the free axis.  Host pre-transposes
    and bf16-casts x; wkv/wgate are fused into one [7168, 512] matrix
    with channel order [kv_lo | sc_lo | kv_hi | sc_hi] and the
    per-phase position embedding (ape) folded in as 4 extra contraction
    rows fed by a 0/1 phase-indicator rhs.
  - The 1024 tokens are processed in 5 chunks (4 x 224 + 128; the short
    last chunk minimizes the exposed final epilogue chain).  Each chunk
    accumulates 57 k-chunks into 2 packed, double-buffered PSUM banks
    (bank A = [kv_lo | sc_lo], bank B = [kv_hi | sc_hi]; only the first
    matmul per bank sets start=True since start clears has_written for
    the whole bank).  The per-chunk epilogue chain (softmax over the 8
    overlap rows via ACT exp + DVE quad-reduces, RMSNorm via
    ones-matmul + broadcast-matmul, rotary via a pair-swap permutation
    matmul, FWHT as one 128x128 matmul) gets a rotating dedicated PSUM
    bank for its small matmul outputs and overlaps the next chunk's
    matmuls.
  - Warm-up matmuls on a zeroed tile bridge the initial DMA wait so the
    PE clock gate is at 8/8 when the real stream starts; x loads ride
    the sync HWDGE queue while weight loads ride the scalar queue.
  Output stays channel-major; host transposes back.
"""

import math
import os

import numpy as np
import ml_dtypes

import concourse.bass as bass
import concourse.bacc as bacc
import concourse.tile as tile
import concourse.mybir as mybir
from concourse.bass_utils import run_bass_kernel_spmd

BF16 = ml_dtypes.bfloat16
F32 = np.float32

# Problem dims (hardcoded per contract)
DIM = 7168
HD = 128
RATIO = 4
COFF = 2
SEQ = 8192
NB = SEQ // RATIO            # 2048 compressed blocks
NCORES = 8
TOK = SEQ // NCORES          # 1024 own tokens per core
NBC = TOK // RATIO           # 256 blocks per core
KC = DIM // 128              # 56 contraction chunks
G = 8                        # k-chunks per w DMA group
NG = KC // G                 # 7 groups
CHUNKS = (512, 224, 224, 64)  # token chunks; chunk 0 unpacked (4 banks),
                              # rest packed (2*qt <= 512 per PSUM bank)
OFFS = (0, 512, 736, 960)
NMISC = 3                    # rotating epilogue PSUM banks
EPS = 1e-6
NEGB = -300.0                # exp(x - 300) == 0.0 in fp32 for masked rows
NDUMMY = 16                  # warm-up matmuls
NBDUMMY = 6                  # boundary gap-filler matmuls (keep HAM warm)

# f32 const pack column layout
C_CD = 0            # cdup [128, 256]
C_SD = 256          # sdup [128, 256]
C_PM = 512          # pmat [128, 128]
C_HM = 640          # hmat [128, 128]
C_NW = 768          # norm_w [128, 1]
C_OK = 769          # ones/HD [128, 1]
C_HB = 770          # halo bias [128, 1]
C_R1 = 771          # all-ones block [128, 128] (row 0 used as [1,128])
C_EPS = 899         # eps (at [0:1, 899])
C_TOT = 900

_cache = {}


def _fwht_mat():
    """fwht(v) = M @ v for the reference's butterfly; fwht(I) = M.T which
    is exactly the lhsT the tensor engine wants."""
    y = np.eye(HD, dtype=np.float64)
    d = HD
    for _ in range(int(math.log2(d))):
        y = y.reshape(y.shape[:-1] + (2, -1))
        a, b = y[..., 0, :], y[..., 1, :]
        y = np.concatenate([a + b, a - b], axis=-1)
    scale = np.float32(d) ** np.float32(-0.5)
    return (y * scale).astype(F32)


def _build_nc():
    nc = bacc.Bacc("TRN2", target_bir_lowering=False)
    f32 = mybir.dt.float32
    bf16 = mybir.dt.bfloat16

    # x, packed chunk-major: rows [(c,g) x 128], cols [cc*qt + t]
    xpA_d = nc.dram_tensor("xpA", [NG * 128, G * 512], bf16, kind="ExternalInput")
    xpB_d = nc.dram_tensor("xpB", [2 * NG * 128, G * 224], bf16, kind="ExternalInput")
    xpC_d = nc.dram_tensor("xpC", [NG * 128, G * 64], bf16, kind="ExternalInput")
    wp_d = nc.dram_tensor("wp", [NG * 128, G * 512], bf16, kind="ExternalInput")
    hx_d = nc.dram_tensor("hx", [128, KC * RATIO], bf16, kind="ExternalInput")
    aug_d = nc.dram_tensor("aug", [RATIO, 1024], bf16, kind="ExternalInput")
    cpk_d = nc.dram_tensor("cpk", [128, C_TOT], f32, kind="ExternalInput")
    out_d = nc.dram_tensor("out", [128, NBC], f32, kind="ExternalOutput")

    AX = mybir.AxisListType
    OP = mybir.AluOpType
    AF = mybir.ActivationFunctionType

    with tile.TileContext(nc) as tc:
        with (
            tc.tile_pool(name="wts", bufs=1) as wts,
            tc.tile_pool(name="csts", bufs=1) as csts,
            tc.tile_pool(name="xs", bufs=10) as xs,
            tc.tile_pool(name="epi", bufs=2) as epi,
            tc.tile_pool(name="ps", bufs=2, space="PSUM") as ps,
            tc.tile_pool(name="mps", bufs=1, space="PSUM") as mps,
        ):
            # rotating PSUM banks for the epilogue's small matmul outputs
            # (+ halo in misc[1], clear of chunk-1's varsum region):
            # [0:nloc bcast][128:+nloc perm][256:+nloc fwht][384:+nloc varsum]
            miscs = [
                mps.tile([128, 512], mybir.dt.float32, name=f"misc{i}", tag=f"misc{i}")
                for i in range(NMISC)
            ]

            hxsb = csts.tile([128, KC * RATIO], bf16, name="hxsb", tag="hxsb")
            nc.gpsimd.dma_start(out=hxsb, in_=hx_d[:, :])
            aug = csts.tile([RATIO, 1024], bf16, name="aug", tag="aug")
            nc.gpsimd.dma_start(out=aug, in_=aug_d[:, :])
            cpk = csts.tile([128, C_TOT], f32, name="cpk", tag="cpk")
            nc.gpsimd.dma_start(out=cpk, in_=cpk_d[:, :])
            waug = aug[:, 0:512]
            xaug = aug[:, 512:1024]
            cdup = cpk[:, C_CD:C_CD + NBC]
            sdup = cpk[:, C_SD:C_SD + NBC]
            pmat = cpk[:, C_PM:C_PM + 128]
            hmat = cpk[:, C_HM:C_HM + 128]
            normw = cpk[:, C_NW:C_NW + 1]
            onesk = cpk[:, C_OK:C_OK + 1]
            hbias = cpk[:, C_HB:C_HB + 1]
            row1 = cpk[0:1, C_R1:C_R1 + 128]
            epsap = cpk[0:1, C_EPS:C_EPS + 1]

            halo_sb = csts.tile([128, 2 * RATIO], f32, name="halo_sb", tag="halo_sb")
            outsb = csts.tile([128, NBC], f32, name="outsb", tag="outsb")

            # PE warm-up on a zeroed tile while the first loads stream in.
            zt = csts.tile([128, 512], bf16, name="zt", tag="zt")
            nc.vector.memset(zt, 0.0)
            for i in range(NDUMMY):
                nc.tensor.matmul(miscs[-1][:, :], zt[:, 0:128], zt[:, :],
                                 start=True, stop=True)

            # ---- weight loads, interleaved with chunk-0 x in need order ----
            wt = []
            for g in range(NG):
                wtg = wts.tile([128, G * 512], bf16, name=f"wt{g}", tag=f"wt{g}")
                wt.append(wtg)

            # ---- x loads on the sync HWDGE queue, chunk-major ----
            def load_xq(ci, g):
                qt = CHUNKS[ci]
                if ci == 0:
                    xq = xs.tile([128, G * 512], bf16, name=f"xq{ci}{g}",
                                 tag="xqbig", bufs=3)
                    src, r0 = xpA_d, g * 128
                else:
                    xq = xs.tile([128, G * 224], bf16, name=f"xq{ci}{g}",
                                 tag="xqs", bufs=7)
                    xq = xq[:, 0:G * qt]
                    src, r0 = (xpB_d, ((ci - 1) * NG + g) * 128) if ci < 3 \
                        else (xpC_d, g * 128)
                if ci == 0:
                    # interleave with the matching weight group, split for
                    # fast first arrival (subtile deps let matmuls start on
                    # the first pieces)
                    step = 1024 if g == 0 else 2048
                    for a in range(0, G * qt, step):
                        nc.sync.dma_start(out=wt[g][:, a:a + step],
                                          in_=wp_d[128 * g:128 * (g + 1), a:a + step])
                        nc.sync.dma_start(out=xq[:, a:a + step],
                                          in_=src[r0:r0 + 128, a:a + step])
                else:
                    nc.sync.dma_start(out=xq, in_=src[r0:r0 + 128, 0:G * qt])
                return xq

            xq_pre = {}

            def chunk_matmuls(ci, outs, packed):
                """57-k-chunk accumulation for CHUNKS[ci] tokens.  For packed
                banks, start=True clears has_written for the WHOLE bank, so
                only the first matmul per bank sets it; the second m-group in
                each bank accumulates cleanly (guarded by PE program order)."""
                qt = CHUNKS[ci]
                for g in range(NG):
                    xq = xq_pre.pop((ci, g), None)
                    if xq is None:
                        xq = load_xq(ci, g)
                    if ci + 1 < len(CHUNKS) and g == NG - 1:
                        # prefetch next chunk's first groups
                        for gg in range(NG):
                            xq_pre[(ci + 1, gg)] = load_xq(ci + 1, gg)
                    for cc in range(G):
                        first = g == 0 and cc == 0
                        for m in range(4):
                            nc.tensor.matmul(
                                outs[m],
                                wt[g][:, cc * 512 + 128 * m:cc * 512 + 128 * (m + 1)],
                                xq[:, cc * qt:(cc + 1) * qt],
                                start=first and (m % 2 == 0 if packed else True),
                                stop=False,
                                skip_group_check=True,
                            )
                    if ci == 0:
                        # halo mini-matmuls ride the k-group whose weights are
                        # resident; keeps PE activity dense.
                        hp = miscs[1][:, 448:456]
                        for cc in range(G):
                            c = g * G + cc
                            for m in range(2):
                                nc.tensor.matmul(
                                    hp[:, 4 * m:4 * (m + 1)],
                                    wt[g][:, cc * 512 + 128 * m:cc * 512 + 128 * (m + 1)],
                                    hxsb[:, 4 * c:4 * (c + 1)],
                                    start=(c == 0 and m == 0),
                                    stop=False,
                                    skip_group_check=True,
                                )
                for m in range(4):
                    nc.tensor.matmul(
                        outs[m],
                        waug[:, 128 * m:128 * (m + 1)],
                        xaug[:, 0:qt],
                        start=False,
                        stop=(m % 2 == 1 if packed else True),
                        skip_group_check=True,
                    )
                if ci == 0:
                    hp = miscs[1][:, 448:456]
                    for m in range(2):
                        nc.tensor.matmul(
                            hp[:, 4 * m:4 * (m + 1)],
                            waug[:, 128 * m:128 * (m + 1)],
                            xaug[:, 0:RATIO],
                            start=False,
                            stop=(m == 1),
                            skip_group_check=True,
                        )
                    nc.scalar.copy(out=halo_sb, in_=hp)

            def epilogue(ci, psums, prev, carry):
                """Softmax+RMS+rotary+FWHT for the blocks of chunk ci."""
                qt = CHUNKS[ci]
                nloc = qt // RATIO
                b0 = OFFS[ci] // RATIO
                kv1p, sc1p, kv2p, sc2p = psums
                misc = miscs[ci % NMISC]
                if carry is not None:
                    nc.scalar.copy(out=carry[:, 0:RATIO], in_=kv1p[:, qt - 4:qt])
                    nc.scalar.copy(out=carry[:, RATIO:2 * RATIO],
                                   in_=sc1p[:, qt - 4:qt])

                E = epi.tile([128, 1024], mybir.dt.float32, name=f"E{ci}", tag="E")
                M = epi.tile([128, 1024], mybir.dt.float32, name=f"M{ci}", tag="M")
                E = E[:, 0:2 * qt]
                M = M[:, 0:2 * qt]
                nc.scalar.activation(
                    E[:, 0:4], prev[:, RATIO:2 * RATIO], AF.Exp,
                    bias=(hbias if ci == 0 else 0.0),
                )
                nc.scalar.activation(E[:, 4:qt], sc1p[:, 0:qt - 4], AF.Exp)
                nc.scalar.activation(E[:, qt:2 * qt], sc2p[:, :], AF.Exp)
                nc.vector.tensor_mul(M[:, 0:4], E[:, 0:4], prev[:, 0:RATIO])
                nc.vector.tensor_mul(M[:, 4:qt], E[:, 4:qt], kv1p[:, 0:qt - 4])
                nc.vector.tensor_mul(M[:, qt:2 * qt], E[:, qt:2 * qt], kv2p[:, :])

                Z = epi.tile([128, 128], mybir.dt.float32, name=f"Z{ci}", tag="Z")
                Z = Z[:, 0:nloc]
                nc.vector.tensor_reduce(
                    Z, E.rearrange("p (t n q) -> p n t q", t=2, q=RATIO),
                    axis=AX.XY, op=OP.add)
                A = epi.tile([128, 128], mybir.dt.float32, name=f"A{ci}", tag="A")
                A = A[:, 0:nloc]
                nc.vector.tensor_reduce(
                    A, M.rearrange("p (t n q) -> p n t q", t=2, q=RATIO),
                    axis=AX.XY, op=OP.add)
                Zr = epi.tile([128, 128], mybir.dt.float32, name=f"Zr{ci}", tag="Zr")
                Zr = Zr[:, 0:nloc]
                nc.vector.reciprocal(Zr, Z)
                comp = epi.tile([128, 128], mybir.dt.float32, name=f"cp{ci}", tag="cp")
                comp = comp[:, 0:nloc]
                nc.vector.tensor_mul(comp, A, Zr)
                sq = epi.tile([128, 128], mybir.dt.float32, name=f"sq{ci}", tag="sq")
                sq = sq[:, 0:nloc]
                nc.vector.tensor_mul(sq, comp, comp)

                nc.tensor.matmul(misc[0:1, 384:384 + nloc], onesk, sq[:, :],
                                 start=True, stop=True)
                sd = epi.tile([1, 128], mybir.dt.float32, name=f"sd{ci}", tag="sd")
                sd = sd[:, 0:nloc]
                nc.scalar.activation(sd, misc[0:1, 384:384 + nloc], AF.Sqrt,
                                     bias=epsap)
                rs = epi.tile([1, 128], mybir.dt.float32, name=f"rs{ci}", tag="rs")
                rs = rs[:, 0:nloc]
                nc.vector.reciprocal(rs, sd)
                nc.tensor.matmul(misc[:, 0:nloc], row1, rs[:, :],
                                 start=True, stop=True)
                compn = epi.tile([128, 128], mybir.dt.float32, name=f"cn{ci}", tag="cn")
                compn = compn[:, 0:nloc]
                nc.vector.scalar_tensor_tensor(
                    out=compn, in0=comp, scalar=normw, in1=misc[:, 0:nloc],
                    op0=OP.mult, op1=OP.mult)
                # rotary: rot = compn*cdup + P @ (compn*sdupP), with sdupP the
                # pair-permuted sin table (host-prepared) so the sin multiply
                # happens BEFORE the permutation matmul (one less serial stage)
                t1 = epi.tile([128, 128], mybir.dt.float32, name=f"t1{ci}", tag="t1")
                t1 = t1[:, 0:nloc]
                nc.vector.tensor_mul(t1, compn, sdup[:, b0:b0 + nloc])
                t2 = epi.tile([128, 128], mybir.dt.float32, name=f"t2{ci}", tag="t2")
                t2 = t2[:, 0:nloc]
                nc.vector.tensor_mul(t2, compn, cdup[:, b0:b0 + nloc])
                nc.tensor.matmul(misc[:, 128:128 + nloc], pmat, t1[:, :],
                                 start=True, stop=True)
                rot = epi.tile([128, 128], mybir.dt.float32, name=f"rt{ci}", tag="rt")
                rot = rot[:, 0:nloc]
                nc.vector.tensor_add(rot, misc[:, 128:128 + nloc], t2)
                nc.tensor.matmul(misc[:, 256:256 + nloc], hmat, rot[:, :],
                                 start=True, stop=True)
                nc.scalar.copy(out=outsb[:, b0:b0 + nloc],
                               in_=misc[:, 256:256 + nloc])
                nc.scalar.dma_start(out=out_d[:, b0:b0 + nloc],
                                    in_=outsb[:, b0:b0 + nloc])

            prev = halo_sb
            for ci, qt in enumerate(CHUNKS):
                if ci == 0:
                    # unpacked: one full bank per m-group, consuming both
                    # slots of each packed tag
                    kv1 = ps.tile([128, 512], mybir.dt.float32,
                                  name="c0kv1", tag="bankA")
                    sc1 = ps.tile([128, 512], mybir.dt.float32,
                                  name="c0sc1", tag="bankA")
                    kv2 = ps.tile([128, 512], mybir.dt.float32,
                                  name="c0kv2", tag="bankB")
                    sc2 = ps.tile([128, 512], mybir.dt.float32,
                                  name="c0sc2", tag="bankB")
                    psums = (kv1, sc1, kv2, sc2)
                    chunk_matmuls(ci, psums, packed=False)
                else:
                    if ci == 1:
                        # gap fillers: keep the PE busy/warm while chunk 0's
                        # epilogue head releases the PSUM slots
                        for i in range(NBDUMMY):
                            nc.tensor.matmul(miscs[2][:, :], zt[:, 0:128],
                                             zt[:, :], start=True, stop=True)
                    bankA = ps.tile([128, 512], mybir.dt.float32,
                                    name=f"bankA{ci}", tag="bankA")
                    bankB = ps.tile([128, 512], mybir.dt.float32,
                                    name=f"bankB{ci}", tag="bankB")
                    psums = (bankA[:, 0:qt], bankA[:, qt:2 * qt],
                             bankB[:, 0:qt], bankB[:, qt:2 * qt])
                    chunk_matmuls(ci, psums, packed=True)
                carry = None
                if ci < len(CHUNKS) - 1:
                    carry = csts.tile([128, 2 * RATIO], f32, name=f"carry{ci}",
                                      tag=f"carry{ci}")
                epilogue(ci, psums, prev, carry)
                prev = carry

    nc.finalize()
    return nc


def _prep_inputs(x, ape, wkv_w, wgate_w, norm_w, cos, sin):
    """Host-side packing of per-core input maps."""
    x = np.asarray(x, dtype=F32)[0]          # [SEQ, DIM]
    ape = np.asarray(ape, dtype=F32)         # [RATIO, 256]
    wkv_w = np.asarray(wkv_w, dtype=F32)     # [256, DIM]
    wgate_w = np.asarray(wgate_w, dtype=F32)
    norm_w = np.asarray(norm_w, dtype=F32)   # [HD]
    cos = np.asarray(cos, dtype=F32)         # [NB, 32]
    sin = np.asarray(sin, dtype=F32)

    xb = x.astype(BF16)

    w_comb = np.concatenate(
        [wkv_w[0:128], wgate_w[0:128], wkv_w[128:256], wgate_w[128:256]], axis=0
    )  # [512, DIM]
    wp = (
        w_comb.T.reshape(NG, G, 128, 512)
        .transpose(0, 2, 1, 3)
        .reshape(NG * 128, G * 512)
        .astype(BF16)
    )
    wp = np.ascontiguousarray(wp)

    # aug: [waug | xaug] as one [4, 1024] bf16 tensor
    aug = np.zeros((RATIO, 1024), dtype=F32)
    aug[:, 0:128] = ape[:, 0:128]
    aug[:, 256:384] = ape[:, 128:256]
    for p in range(RATIO):
        aug[p, 512 + p::RATIO] = 1.0
    aug = aug.astype(BF16)

    pmat = np.zeros((128, 128), dtype=F32)
    for i in range(32):
        pmat[2 * i, 2 * i + 1] = 1.0
        pmat[2 * i + 1, 2 * i] = 1.0
    hmat = _fwht_mat()

    in_maps = []
    for c in range(NCORES):
        t0 = c * TOK
        seg = xb[t0:t0 + TOK]                       # [1024, DIM]
        segT = np.ascontiguousarray(seg.T)          # [DIM, 1024]
        kview = segT.reshape(KC, 128, TOK)          # [kchunk, p, t]

        def pack_chunk(ci):
            qt, o = CHUNKS[ci], OFFS[ci]
            # [NG, 128, G*qt]: row (g,p), col cc*qt+t = kview[8g+cc, p, o+t]
            a = kview[:, :, o:o + qt]               # [56, 128, qt]
            a = a.reshape(NG, G, 128, qt).transpose(0, 2, 1, 3)
            return a.reshape(NG * 128, G * qt)

        xpA = np.ascontiguousarray(pack_chunk(0))
        xpB = np.ascontiguousarray(
            np.concatenate([pack_chunk(1), pack_chunk(2)], axis=0))
        xpC = np.ascontiguousarray(pack_chunk(3))

        if c == 0:
            halo = np.zeros((RATIO, DIM), dtype=BF16)
        else:
            halo = xb[t0 - RATIO:t0]
        hx = np.ascontiguousarray(
            halo.T.reshape(KC, 128, RATIO).transpose(1, 0, 2).reshape(128, KC * RATIO)
        )

        b0 = c * NBC
        cs = cos[b0:b0 + NBC]                       # [NBC, 32]
        ss = sin[b0:b0 + NBC]
        cpk = np.zeros((128, C_TOT), dtype=F32)
        cd = np.ones((128, NBC), dtype=F32)
        sd = np.zeros((128, NBC), dtype=F32)
        cd[0:64:2] = cs.T
        cd[1:64:2] = cs.T
        # pair-permuted sin table: the sin multiply happens before the
        # pair-swap matmul, so sdupP[h] = sdup_orig[h^1]
        sd[0:64:2] = ss.T
        sd[1:64:2] = -ss.T
        cpk[:, C_CD:C_CD + NBC] = cd
        cpk[:, C_SD:C_SD + NBC] = sd
        cpk[:, C_PM:C_PM + 128] = pmat
        cpk[:, C_HM:C_HM + 128] = hmat
        cpk[:, C_NW] = norm_w
        cpk[:, C_OK] = 1.0 / HD
        cpk[:, C_HB] = NEGB if c == 0 else 0.0
        cpk[:, C_R1:C_R1 + 128] = 1.0
        cpk[0, C_EPS] = EPS

        in_maps.append(dict(xpA=xpA, xpB=xpB, xpC=xpC, wp=wp, hx=hx, aug=aug,
                            cpk=np.ascontiguousarray(cpk)))
    return in_maps


LAST_RESULTS = None


def kernel(x, ape, wkv_w, wgate_w, norm_w, cos, sin, start_pos=0,
           compress_state=None, **_unused):
    global LAST_RESULTS
    in_maps = _prep_inputs(x, ape, wkv_w, wgate_w, norm_w, cos, sin)
    if "nc" not in _cache:
        _cache["nc"] = _build_nc()
    nc = _cache["nc"]
    trace = bool(int(os.environ.get("KERNEL_TRACE", "0") or 0))
    tdir = os.environ.get("KERNEL_TRACE_DIR") or None
    res = run_bass_kernel_spmd(
        nc, in_maps, core_ids=list(range(NCORES)),
        trace=trace,
        trace_cores=[0] if trace else None,
        tmpdir=tdir,
    )
    LAST_RESULTS = res
    out = np.empty((1, NB, HD), dtype=F32)
    for c in range(NCORES):
        out[0, c * NBC:(c + 1) * NBC, :] = res.results[c]["out"].T
    return out

